# revision 17
# baseline (speedup 1.0000x reference)
"""Trainium2 Bass kernel for nn_MultiHeadCrossAttention_78503412236770.

Math (validated against the jax reference to ~1e-6 rel):
  The inner softmax is over a SINGLETON axis -> attn == 1 exactly.
  Therefore:
    attn_out = ones(B, H, NK)
    out      = kp.sum(axis=NK)  which commutes with the key projection:
               ksum = k.sum(1); out_pre = ksum @ Wk.T + NK*bk  (head reshuffles cancel)
    out2     = out_pre @ Wo.T + bo = ksum @ (Wo@Wk).T + (NK*(Wo@bk) + bo)
    out3     = x_hid @ Wo.T + bo with x_hid = perm(latent),
               latent = BN2(perm(BN1(x) @ Wx.T + bx)) @ Wlin.T + blin
  So the only large-tensor work is ksum = k.sum(axis=1): a 256 MB HBM-bound
  reduction, data-parallel over batch across 8 cores (32 MB/core). BatchNorm
  stats need the full batch, so x (512 KB) is replicated and the small path is
  computed redundantly per core -- no collectives needed.

Implementation notes:
  - fp32 PE matmuls are double-pumped (LOW_HIGH) at 2 cyc/col -> 1/4 bf16
    rate, so the bulk reduction runs on DVE+GPSIMD as pairwise adds instead.
  - k streams as [128, 8, 512] tiles with partition = (batch, n-subblock)
    (uniform 256 KB partition stride, 16 KB contiguous per partition). The
    8 n-chunks per tile are folded by elementwise adds into two running
    accumulators acc[p=(b,ns), d]; at the end one tiny PE matmul against a
    block-diagonal 0/1 matrix G folds the 8 ns per batch and yields ksumT
    [dk, b] directly.
  - All stationary matmul operands are narrow (LDWEIGHTS ~ cols/1.2GHz);
    weights stream as the moving operand at N<=512; row biases enter as
    rank-1 matmuls; per-partition biases via ACT copy bias. Compute-engine
    partition bases only in {0, 64}.
"""

import os
import sys

import numpy as np

for _p in ("/opt/trn_rl_repo", "/root/.axon_site/_ro/trn_rl_repo"):
    if os.path.isdir(_p) and _p not in sys.path:
        sys.path.insert(0, _p)

from contextlib import ExitStack

import concourse.bass as bass
import concourse.tile as tile
from concourse import bacc, mybir
from concourse.bass_utils import run_bass_kernel_spmd
from concourse.masks import make_identity

B, DX, NK, DKIN, H, DK, DO = 128, 1024, 1024, 512, 16, 64, 512
NCORES = 8
BL = B // NCORES  # 16 batch rows per core
NT = 16  # k tiles per core
GPS_TILES = (1, 3, 6, 8, 10, 12)  # tiles reduced on GpSimd (~2x slower than DVE)
EPS = 1e-5
F32 = mybir.dt.float32
AF = mybir.ActivationFunctionType
ALU = mybir.AluOpType


def build_program():
    nc = bacc.Bacc(
        "TRN2",
        target_bir_lowering=False,
        debug=False,
        enable_asserts=True,
        num_devices=NCORES,
    )
    kc = nc.dram_tensor("kc", [BL, NK, DKIN], F32, kind="ExternalInput").ap()
    xT = nc.dram_tensor("xT", [DX, B], F32, kind="ExternalInput").ap()
    WxT = nc.dram_tensor("WxT", [DX, H * DK], F32, kind="ExternalInput").ap()
    WlinT = nc.dram_tensor("WlinT", [DK, DK], F32, kind="ExternalInput").ap()
    WoT = nc.dram_tensor("WoT", [H * DK, DO], F32, kind="ExternalInput").ap()
    WkoT = nc.dram_tensor("WkoT", [DKIN, DO], F32, kind="ExternalInput").ap()
    gmat = nc.dram_tensor("gmat", [128, BL], F32, kind="ExternalInput").ap()
    vp128 = nc.dram_tensor("vp128", [128, 17], F32, kind="ExternalInput").ap()
    vp64 = nc.dram_tensor("vp64", [64, 2], F32, kind="ExternalInput").ap()
    bx_row = nc.dram_tensor("bx_row", [1, H * DK], F32, kind="ExternalInput").ap()
    bo_row = nc.dram_tensor("bo_row", [1, DO], F32, kind="ExternalInput").ap()
    bko_row = nc.dram_tensor("bko_row", [1, DO], F32, kind="ExternalInput").ap()
    attn_o = nc.dram_tensor("attn_o", [BL * H, NK], F32, kind="ExternalOutput").ap()
    out2_o = nc.dram_tensor("out2_o", [BL, DO], F32, kind="ExternalOutput").ap()
    out3_o = nc.dram_tensor("out3_o", [B, DO], F32, kind="ExternalOutput").ap()

    # k tile view: tile t, partition p=(b, ns), free (c, d); n = ns*128 + t*8 + c
    kts = kc.rearrange("b (ns ct c) d -> ct (b ns) c d", ns=8, c=8)

    with tile.TileContext(nc) as tc, ExitStack() as ctx:
        const = ctx.enter_context(tc.tile_pool(name="const", bufs=1))
        wpool = ctx.enter_context(tc.tile_pool(name="weights", bufs=1))
        big = ctx.enter_context(tc.tile_pool(name="big", bufs=1))
        stat = ctx.enter_context(tc.tile_pool(name="stat", bufs=4))
        kpool = ctx.enter_context(tc.tile_pool(name="kpool", bufs=3))
        atp = ctx.enter_context(tc.tile_pool(name="atp", bufs=1))
        outp = ctx.enter_context(tc.tile_pool(name="outp", bufs=1))
        psq = ctx.enter_context(tc.tile_pool(name="psq", bufs=2, space="PSUM"))
        ps = ctx.enter_context(tc.tile_pool(name="ps", bufs=2, space="PSUM"))

        # constants
        onesrow = const.tile([1, 512], F32)
        nc.vector.memset(onesrow, 1.0)
        ones128 = const.tile([128, 1], F32)
        nc.vector.memset(ones128, 1.0)
        epst = const.tile([128, 1], F32)
        nc.vector.memset(epst, EPS)
        ident = const.tile([128, 128], F32)
        make_identity(nc, ident)
        onesA = const.tile([128, 512], F32)
        nc.vector.memset(onesA, 1.0)
        for ai in range(4):
            nc.scalar.dma_start(
                out=attn_o[64 * ai : 64 * (ai + 1), :], in_=onesA
            )

        # small inputs first on the ACT ring so BN1/xq can start early
        xt = [wpool.tile([128, B], F32, tag=f"xt{j}", name=f"xt{j}") for j in range(8)]
        for j in range(8):
            nc.scalar.dma_start(out=xt[j], in_=xT[128 * j : 128 * (j + 1), :])
        vp128_sb = const.tile([128, 17], F32)
        nc.scalar.dma_start(out=vp128_sb, in_=vp128)
        vp64_sb = const.tile([64, 2], F32)
        nc.scalar.dma_start(out=vp64_sb, in_=vp64)
        g_sb = const.tile([128, BL], F32)
        nc.scalar.dma_start(out=g_sb, in_=gmat)
        bxr = const.tile([1, H * DK], F32)
        nc.scalar.dma_start(out=bxr, in_=bx_row)
        bor = const.tile([1, DO], F32)
        nc.scalar.dma_start(out=bor, in_=bo_row)
        bkor = const.tile([1, DO], F32)
        nc.scalar.dma_start(out=bkor, in_=bko_row)
        wx = [wpool.tile([128, H * DK], F32, tag=f"wx{j}", name=f"wx{j}") for j in range(8)]
        for j in range(8):
            nc.scalar.dma_start(out=wx[j], in_=WxT[128 * j : 128 * (j + 1), :])
        wlin = wpool.tile([64, 64], F32)
        nc.scalar.dma_start(out=wlin, in_=WlinT)
        wop = [wpool.tile([128, DO], F32, tag=f"wop{i}", name=f"wop{i}") for i in range(8)]
        for i in range(8):
            nc.scalar.dma_start(out=wop[i], in_=WoT[128 * i : 128 * (i + 1), :])
        wko = [wpool.tile([128, DO], F32, tag=f"wko{g}", name=f"wko{g}") for g in range(4)]
        for g in range(4):
            nc.scalar.dma_start(out=wko[g], in_=WkoT[128 * g : 128 * (g + 1), :])

        # running accumulators acc[p=(b,ns), d], one per reducer engine
        accD = big.tile([128, DKIN], F32)
        accG = big.tile([128, DKIN], F32)
        dve_seen = gps_seen = False

        def ktile(t):
            nonlocal dve_seen, gps_seen
            kt = kpool.tile([128, 8, DKIN], F32, tag="kt", name=f"kt{t}")
            nc.sync.dma_start(out=kt, in_=kts[t])
            if t in GPS_TILES:
                eng, acc, first = nc.gpsimd, accG, not gps_seen
                gps_seen = True
                at = atp.tile([128, 4, DKIN], F32, tag="atG", name=f"atG{t}")
            else:
                eng, acc, first = nc.vector, accD, not dve_seen
                dve_seen = True
                at = atp.tile([128, 4, DKIN], F32, tag="atD", name=f"atD{t}")
            eng.tensor_add(out=at, in0=kt[:, 0:4, :], in1=kt[:, 4:8, :])
            eng.tensor_add(out=at[:, 0:2, :], in0=at[:, 0:2, :], in1=at[:, 2:4, :])
            if first:
                eng.tensor_add(out=acc, in0=at[:, 0, :], in1=at[:, 1, :])
            else:
                eng.tensor_add(out=at[:, 0, :], in0=at[:, 0, :], in1=at[:, 1, :])
                eng.tensor_add(out=acc, in0=acc, in1=at[:, 0, :])

        # head start on the k stream
        for t in range(0, 6):
            ktile(t)

        # ---- small path (redundant on every core) ----
        # BN1 on x.T tiles: per-partition stats over the batch (free) axis
        xnt = [wpool.tile([128, B], F32, tag=f"xnt{j}", name=f"xnt{j}") for j in range(8)]
        for j in range(8):
            st = stat.tile([128, 6], F32)
            nc.vector.bn_stats(out=st, in_=xt[j])
            mv = stat.tile([128, 2], F32)
            nc.vector.bn_aggr(out=mv, in_=st)
            rs = stat.tile([128, 1], F32)
            nc.scalar.activation(out=rs, in_=mv[:, 1:2], func=AF.Sqrt, bias=epst, scale=1.0)
            nc.vector.reciprocal(out=rs, in_=rs)
            s1 = stat.tile([128, 1], F32)
            nc.vector.tensor_mul(out=s1, in0=rs, in1=vp128_sb[:, j : j + 1])
            t1 = stat.tile([128, 1], F32)
            nc.vector.tensor_mul(out=t1, in0=mv[:, 0:1], in1=s1)
            nc.vector.tensor_sub(out=t1, in0=vp128_sb[:, 8 + j : 9 + j], in1=t1)
            nc.vector.tensor_scalar(
                out=xnt[j], in0=xt[j], scalar1=s1, scalar2=t1, op0=ALU.mult, op1=ALU.add
            )

        # xq[b, m] = xn[b, :] @ Wx.T[:, m] + bx[m]: xn.T stationary, Wx.T moving
        xq_sb = big.tile([128, H * DK], F32)
        for half in range(2):
            pq = psq.tile([128, 512], F32, tag="psq")
            for j in range(8):
                nc.tensor.matmul(
                    pq,
                    lhsT=xnt[j],
                    rhs=wx[j][:, 512 * half : 512 * (half + 1)],
                    start=(j == 0),
                    stop=False,
                )
            nc.tensor.matmul(
                pq,
                lhsT=onesrow[:, 0:128],
                rhs=bxr[:, 512 * half : 512 * (half + 1)],
                start=False,
                stop=True,
            )
            nc.scalar.copy(out=xq_sb[:, 512 * half : 512 * (half + 1)], in_=pq)

        # transpose per head-pair: xq2T[dkq, h*B + b] = xq[b, h*DK + dkq]
        xq2T = big.tile([64, H * B], F32)
        for i in range(8):
            ptp = ps.tile([128, 128], F32, tag="ps")
            nc.tensor.transpose(ptp, xq_sb[:, 128 * i : 128 * (i + 1)], ident)
            nc.scalar.copy(out=xq2T[:, 256 * i : 256 * i + 128], in_=ptp[0:64, :])
            nc.scalar.copy(out=xq2T[:, 256 * i + 128 : 256 * i + 256], in_=ptp[64:128, :])

        # rest of the k stream
        for t in range(6, NT):
            ktile(t)

        # BN2 stats over (b, h) per d2, computed in the [b, m] layout:
        # column sums of xq_sb/xq_sb^2 via ones-matmuls, then fold h on 1 lane
        srow = stat.tile([1, 2, H * DK], F32)
        for half in range(2):
            sq_sb = big.tile([128, 512], F32, tag="sqsb", name=f"sq{half}", bufs=2)
            nc.scalar.activation(
                out=sq_sb, in_=xq_sb[:, 512 * half : 512 * (half + 1)],
                func=AF.Square, bias=0.0, scale=1.0,
            )
            psSh = ps.tile([1, 2, 512], F32, tag="psS", bufs=1, name=f"psS{half}")
            nc.tensor.matmul(
                psSh[:, 0, :], lhsT=ones128,
                rhs=xq_sb[:, 512 * half : 512 * (half + 1)], start=True, stop=True,
            )
            nc.tensor.matmul(psSh[:, 1, :], lhsT=ones128, rhs=sq_sb, start=True, stop=True)
            nc.scalar.copy(out=srow[:, :, 512 * half : 512 * (half + 1)], in_=psSh)
        # fold h (stride-DK view, innermost = h) -> [1, 2, DK] sums, then *1/(H*B)
        mrow = stat.tile([1, 2, DK], F32)
        nc.vector.tensor_reduce(
            out=mrow,
            in_=srow.rearrange("p s (h d) -> p s d h", h=H),
            axis=mybir.AxisListType.X, op=ALU.add,
        )
        nc.scalar.mul(out=mrow, in_=mrow, mul=1.0 / (H * B))
        # var = E[x^2] - E[x]^2 on the single lane
        vrow = stat.tile([1, DK], F32)
        nc.vector.tensor_mul(out=vrow, in0=mrow[:, 0, :], in1=mrow[:, 0, :])
        nc.vector.tensor_sub(out=vrow, in0=mrow[:, 1, :], in1=vrow)
        # transpose mean/var rows -> per-partition [64, 1] scalars
        psmv = ps.tile([64, 2], F32, tag="psmv", bufs=1)
        nc.tensor.transpose(psmv[:, 0:1], mrow[:, 0, :], ident[0:1, 0:1])
        nc.tensor.transpose(psmv[:, 1:2], vrow, ident[0:1, 0:1])
        mv2 = stat.tile([64, 2], F32)
        nc.scalar.copy(out=mv2, in_=psmv)
        rs2 = stat.tile([64, 1], F32)
        nc.scalar.activation(
            out=rs2, in_=mv2[:, 1:2], func=AF.Sqrt, bias=epst[0:64, :], scale=1.0
        )
        nc.vector.reciprocal(out=rs2, in_=rs2)
        s2 = stat.tile([64, 1], F32)
        nc.vector.tensor_mul(out=s2, in0=rs2, in1=vp64_sb[:, 0:1])
        t2 = stat.tile([64, 1], F32)
        nc.vector.tensor_mul(out=t2, in0=mv2[:, 0:1], in1=s2)
        nc.vector.tensor_sub(out=t2, in0=vp64_sb[:, 1:2], in1=t2)
        # apply on ACT (per-partition scale+bias), keeping DVE free for the k adds
        nc.scalar.activation(out=xq2T, in_=xq2T, func=AF.Identity, bias=t2, scale=s2)

        # latent in head-pair layout: latp[hl*64+d2, i*128+b] = latent[(2i+hl)*B+b, d2]
        latp = big.tile([128, 8 * B], F32)
        for half in range(2):
            psl = psq.tile([128, 512], F32, tag="psq")
            for hh in range(4):
                h = half * 8 + hh * 2
                for hl in range(2):
                    nc.tensor.matmul(
                        psl[64 * hl : 64 * hl + 64, 128 * hh : 128 * (hh + 1)],
                        lhsT=wlin,
                        rhs=xq2T[:, B * (h + hl) : B * (h + hl + 1)],
                        start=True,
                        stop=True,
                    )
            # blin (duplicated per half-pair) via ACT copy bias
            nc.scalar.activation(
                out=latp[:, 512 * half : 512 * (half + 1)],
                in_=psl,
                func=AF.Identity,
                bias=vp128_sb[:, 16:17],
                scale=1.0,
            )

        # out3[b, o] = sum_i latp[:, i*128:+128].T @ WoT[128i:+128, :] + bo
        ps3 = psq.tile([128, DO], F32, tag="psq")
        for i in range(8):
            nc.tensor.matmul(
                ps3,
                lhsT=latp[:, 128 * i : 128 * (i + 1)],
                rhs=wop[i],
                start=(i == 0),
                stop=False,
            )
        nc.tensor.matmul(ps3, lhsT=onesrow[:, 0:128], rhs=bor, start=False, stop=True)
        out3_sb = outp.tile([128, DO], F32, tag="out3sb")
        nc.scalar.copy(out=out3_sb, in_=ps3)
        nc.scalar.dma_start(out=out3_o, in_=out3_sb)

        # fold ns per batch: ksumT[dk, b] = sum_p (accD+accG)[p, dk] * G[p, b]
        psT = psq.tile([128, 4, BL], F32, tag="psq")
        for g in range(4):
            nc.tensor.matmul(
                psT[:, g, :], lhsT=accD[:, 128 * g : 128 * (g + 1)], rhs=g_sb,
                start=True, stop=False,
            )
            nc.tensor.matmul(
                psT[:, g, :], lhsT=accG[:, 128 * g : 128 * (g + 1)], rhs=g_sb,
                start=False, stop=True,
            )
        ksumT = big.tile([128, 4, BL], F32)
        nc.scalar.copy(out=ksumT, in_=psT)

        # out2 = ksum @ Wko.T + bko
        ps2 = psq.tile([BL, DO], F32, tag="psq")
        for g in range(4):
            nc.tensor.matmul(
                ps2, lhsT=ksumT[:, g, :], rhs=wko[g], start=(g == 0), stop=False
            )
        nc.tensor.matmul(ps2, lhsT=onesrow[:, 0:BL], rhs=bkor, start=False, stop=True)
        out2_sb = outp.tile([BL, DO], F32, tag="out2sb")
        nc.scalar.copy(out=out2_sb, in_=ps2)
        nc.scalar.dma_start(out=out2_o, in_=out2_sb)

    nc.compile()
    return nc


_CACHE = {"nc": None}


def get_nc():
    if _CACHE["nc"] is None:
        _CACHE["nc"] = build_program()
    return _CACHE["nc"]


def make_in_maps(inputs):
    f = lambda a: np.ascontiguousarray(np.asarray(a, dtype=np.float32))
    x, k = f(inputs["x"]), np.asarray(inputs["k"], dtype=np.float32)
    bn1_g, bn1_b = f(inputs["bn1_g"]), f(inputs["bn1_b"])
    Wx, bx = f(inputs["Wx"]), f(inputs["bx"])
    Wk, bk = f(inputs["Wk"]), f(inputs["bk"])
    bn2_g, bn2_b = f(inputs["bn2_g"]), f(inputs["bn2_b"])
    Wlin, blin = f(inputs["Wlin"]), f(inputs["blin"])
    Wo, bo = f(inputs["Wo"]), f(inputs["bo"])

    bko = (np.float32(NK) * (Wo @ bk) + bo).astype(np.float32)
    vp128v = np.zeros((128, 17), np.float32)
    vp128v[:, 0:8] = bn1_g.reshape(8, 128).T
    vp128v[:, 8:16] = bn1_b.reshape(8, 128).T
    vp128v[:, 16] = np.concatenate([blin, blin])
    vp64v = np.zeros((64, 2), np.float32)
    vp64v[:, 0] = bn2_g
    vp64v[:, 1] = bn2_b
    gmatv = (np.arange(128)[:, None] // 8 == np.arange(BL)[None, :]).astype(np.float32)

    shared = {
        "xT": np.ascontiguousarray(x.T),
        "WxT": np.ascontiguousarray(Wx.T),
        "WlinT": np.ascontiguousarray(Wlin.T),
        "WoT": np.ascontiguousarray(Wo.T),
        "WkoT": np.ascontiguousarray((Wo @ Wk).T),
        "gmat": gmatv,
        "vp128": vp128v,
        "vp64": vp64v,
        "bx_row": bx.reshape(1, H * DK),
        "bo_row": bo.reshape(1, DO),
        "bko_row": bko.reshape(1, DO),
    }
    return [
        {**shared, "kc": np.ascontiguousarray(k[c * BL : (c + 1) * BL])}
        for c in range(NCORES)
    ]


def gather_outputs(results):
    attn = np.concatenate(
        [r["attn_o"].reshape(BL, H, NK) for r in results], axis=0
    ).astype(np.float32)
    out2 = np.concatenate([r["out2_o"] for r in results], axis=0).astype(np.float32)
    out3 = results[0]["out3_o"].astype(np.float32)
    return attn, out2, out3


def kernel(**inputs):
    nc = get_nc()
    in_maps = make_in_maps(inputs)
    res = run_bass_kernel_spmd(nc, in_maps, core_ids=list(range(NCORES)))
    return gather_outputs(res.results)


# revision 19
# speedup vs baseline: 1.0814x; 1.0814x over previous
"""Trainium2 Bass kernel for nn_MultiHeadCrossAttention_78503412236770.

Math (validated against the jax reference to ~1e-6 rel):
  The inner softmax is over a SINGLETON axis -> attn == 1 exactly.
  Therefore:
    attn_out = ones(B, H, NK)
    out      = kp.sum(axis=NK)  which commutes with the key projection:
               ksum = k.sum(1); out_pre = ksum @ Wk.T + NK*bk  (head reshuffles cancel)
    out2     = out_pre @ Wo.T + bo = ksum @ (Wo@Wk).T + (NK*(Wo@bk) + bo)
    out3     = x_hid @ Wo.T + bo with x_hid = perm(latent),
               latent = BN2(perm(BN1(x) @ Wx.T + bx)) @ Wlin.T + blin
  So the only large-tensor work is ksum = k.sum(axis=1): a 256 MB HBM-bound
  reduction, data-parallel over batch across 8 cores (32 MB/core). BatchNorm
  stats need the full batch, so x (512 KB) is replicated and the small path is
  computed redundantly per core -- no collectives needed.

Implementation notes:
  - fp32 PE matmuls are double-pumped (LOW_HIGH) at 2 cyc/col -> 1/4 bf16
    rate, so the bulk reduction runs on DVE+GPSIMD as pairwise adds instead.
  - k streams as [128, 8, 512] tiles with partition = (batch, n-subblock)
    (uniform 256 KB partition stride, 16 KB contiguous per partition). The
    8 n-chunks per tile are folded by elementwise adds into two running
    accumulators acc[p=(b,ns), d]; at the end one tiny PE matmul against a
    block-diagonal 0/1 matrix G folds the 8 ns per batch and yields ksumT
    [dk, b] directly.
  - All stationary matmul operands are narrow (LDWEIGHTS ~ cols/1.2GHz);
    weights stream as the moving operand at N<=512; row biases enter as
    rank-1 matmuls; per-partition biases via ACT copy bias. Compute-engine
    partition bases only in {0, 64}.
"""

import os
import sys

import numpy as np

for _p in ("/opt/trn_rl_repo", "/root/.axon_site/_ro/trn_rl_repo"):
    if os.path.isdir(_p) and _p not in sys.path:
        sys.path.insert(0, _p)

from contextlib import ExitStack

import concourse.bass as bass
import concourse.tile as tile
from concourse import bacc, mybir
from concourse.bass_utils import run_bass_kernel_spmd
from concourse.masks import make_identity

B, DX, NK, DKIN, H, DK, DO = 128, 1024, 1024, 512, 16, 64, 512
NCORES = 8
BL = B // NCORES  # 16 batch rows per core
NT = 16  # k tiles per core
GPS_REST_TILES = tuple(range(12))  # chain tail on GpSimd for these tiles
EPS = 1e-5
F32 = mybir.dt.float32
AF = mybir.ActivationFunctionType
ALU = mybir.AluOpType


def build_program():
    nc = bacc.Bacc(
        "TRN2",
        target_bir_lowering=False,
        debug=False,
        enable_asserts=True,
        num_devices=NCORES,
    )
    kc = nc.dram_tensor("kc", [BL, NK, DKIN], F32, kind="ExternalInput").ap()
    xT = nc.dram_tensor("xT", [DX, B], F32, kind="ExternalInput").ap()
    WxT = nc.dram_tensor("WxT", [DX, H * DK], F32, kind="ExternalInput").ap()
    WlinT = nc.dram_tensor("WlinT", [DK, DK], F32, kind="ExternalInput").ap()
    WoT = nc.dram_tensor("WoT", [H * DK, DO], F32, kind="ExternalInput").ap()
    WkoT = nc.dram_tensor("WkoT", [DKIN, DO], F32, kind="ExternalInput").ap()
    gmat = nc.dram_tensor("gmat", [128, BL], F32, kind="ExternalInput").ap()
    vp128 = nc.dram_tensor("vp128", [128, 17], F32, kind="ExternalInput").ap()
    vp64 = nc.dram_tensor("vp64", [64, 2], F32, kind="ExternalInput").ap()
    bx_row = nc.dram_tensor("bx_row", [1, H * DK], F32, kind="ExternalInput").ap()
    bo_row = nc.dram_tensor("bo_row", [1, DO], F32, kind="ExternalInput").ap()
    bko_row = nc.dram_tensor("bko_row", [1, DO], F32, kind="ExternalInput").ap()
    attn_o = nc.dram_tensor("attn_o", [BL * H, NK], F32, kind="ExternalOutput").ap()
    out2_o = nc.dram_tensor("out2_o", [BL, DO], F32, kind="ExternalOutput").ap()
    out3_o = nc.dram_tensor("out3_o", [B, DO], F32, kind="ExternalOutput").ap()

    # k tile view: tile t, partition p=(b, ns), free (c, d); n = ns*128 + t*8 + c
    kts = kc.rearrange("b (ns ct c) d -> ct (b ns) c d", ns=8, c=8)

    with tile.TileContext(nc) as tc, ExitStack() as ctx:
        const = ctx.enter_context(tc.tile_pool(name="const", bufs=1))
        wpool = ctx.enter_context(tc.tile_pool(name="weights", bufs=1))
        big = ctx.enter_context(tc.tile_pool(name="big", bufs=1))
        stat = ctx.enter_context(tc.tile_pool(name="stat", bufs=3))
        kpool = ctx.enter_context(tc.tile_pool(name="kpool", bufs=3))
        atp = ctx.enter_context(tc.tile_pool(name="atp", bufs=3))
        outp = ctx.enter_context(tc.tile_pool(name="outp", bufs=1))
        psq = ctx.enter_context(tc.tile_pool(name="psq", bufs=2, space="PSUM"))
        ps = ctx.enter_context(tc.tile_pool(name="ps", bufs=2, space="PSUM"))

        # constants
        onesrow = const.tile([1, 512], F32)
        nc.vector.memset(onesrow, 1.0)
        ones128 = const.tile([128, 1], F32)
        nc.vector.memset(ones128, 1.0)
        epst = const.tile([128, 1], F32)
        nc.vector.memset(epst, EPS)
        ident = const.tile([128, 128], F32)
        make_identity(nc, ident)
        onesA = const.tile([128, 512], F32)
        nc.vector.memset(onesA, 1.0)
        for ai in range(4):
            nc.scalar.dma_start(
                out=attn_o[64 * ai : 64 * (ai + 1), :], in_=onesA
            )

        # small inputs first on the ACT ring so BN1/xq can start early
        xt = [wpool.tile([128, B], F32, tag=f"xt{j}", name=f"xt{j}") for j in range(8)]
        for j in range(8):
            nc.scalar.dma_start(out=xt[j], in_=xT[128 * j : 128 * (j + 1), :])
        vp128_sb = const.tile([128, 17], F32)
        nc.scalar.dma_start(out=vp128_sb, in_=vp128)
        vp64_sb = const.tile([64, 2], F32)
        nc.scalar.dma_start(out=vp64_sb, in_=vp64)
        g_sb = const.tile([128, BL], F32)
        nc.scalar.dma_start(out=g_sb, in_=gmat)
        bxr = const.tile([1, H * DK], F32)
        nc.scalar.dma_start(out=bxr, in_=bx_row)
        bor = const.tile([1, DO], F32)
        nc.scalar.dma_start(out=bor, in_=bo_row)
        bkor = const.tile([1, DO], F32)
        nc.scalar.dma_start(out=bkor, in_=bko_row)
        wx = [wpool.tile([128, H * DK], F32, tag=f"wx{j}", name=f"wx{j}") for j in range(8)]
        for j in range(8):
            nc.scalar.dma_start(out=wx[j], in_=WxT[128 * j : 128 * (j + 1), :])
        wlin = wpool.tile([64, 64], F32)
        nc.scalar.dma_start(out=wlin, in_=WlinT)
        wop = [wpool.tile([128, DO], F32, tag=f"wop{i}", name=f"wop{i}") for i in range(8)]
        for i in range(8):
            nc.scalar.dma_start(out=wop[i], in_=WoT[128 * i : 128 * (i + 1), :])
        wko = [wpool.tile([128, DO], F32, tag=f"wko{g}", name=f"wko{g}") for g in range(4)]
        for g in range(4):
            nc.scalar.dma_start(out=wko[g], in_=WkoT[128 * g : 128 * (g + 1), :])

        # running accumulators acc[p=(b,ns), d], one per reducer engine
        accD = big.tile([128, DKIN], F32)
        accG = big.tile([128, DKIN], F32)
        dve_seen = gps_seen = False

        def ktile(t):
            nonlocal dve_seen, gps_seen
            kt = kpool.tile([128, 8, DKIN], F32, tag="kt", name=f"kt{t}")
            nc.sync.dma_start(out=kt, in_=kts[t])
            at = atp.tile([128, 4, DKIN], F32, tag="at", name=f"at{t}")
            # add1 always on DVE: frees the kt slot fast so the stream never
            # stalls behind the slower GpSimd
            nc.vector.tensor_add(out=at, in0=kt[:, 0:4, :], in1=kt[:, 4:8, :])
            if t in GPS_REST_TILES:
                eng, acc, first = nc.gpsimd, accG, not gps_seen
                gps_seen = True
            else:
                eng, acc, first = nc.vector, accD, not dve_seen
                dve_seen = True
            eng.tensor_add(out=at[:, 0:2, :], in0=at[:, 0:2, :], in1=at[:, 2:4, :])
            if first:
                eng.tensor_add(out=acc, in0=at[:, 0, :], in1=at[:, 1, :])
            else:
                eng.tensor_add(out=at[:, 0, :], in0=at[:, 0, :], in1=at[:, 1, :])
                eng.tensor_add(out=acc, in0=acc, in1=at[:, 0, :])

        # head start on the k stream
        for t in range(0, 6):
            ktile(t)

        # ---- small path (redundant on every core) ----
        # BN1 on x.T tiles: per-partition stats over the batch (free) axis
        xnt = [wpool.tile([128, B], F32, tag=f"xnt{j}", name=f"xnt{j}") for j in range(8)]
        for j in range(8):
            st = stat.tile([128, 6], F32)
            nc.vector.bn_stats(out=st, in_=xt[j])
            mv = stat.tile([128, 2], F32)
            nc.vector.bn_aggr(out=mv, in_=st)
            rs = stat.tile([128, 1], F32)
            nc.scalar.activation(out=rs, in_=mv[:, 1:2], func=AF.Sqrt, bias=epst, scale=1.0)
            nc.vector.reciprocal(out=rs, in_=rs)
            s1 = stat.tile([128, 1], F32)
            nc.vector.tensor_mul(out=s1, in0=rs, in1=vp128_sb[:, j : j + 1])
            t1 = stat.tile([128, 1], F32)
            nc.vector.tensor_mul(out=t1, in0=mv[:, 0:1], in1=s1)
            nc.vector.tensor_sub(out=t1, in0=vp128_sb[:, 8 + j : 9 + j], in1=t1)
            nc.vector.tensor_scalar(
                out=xnt[j], in0=xt[j], scalar1=s1, scalar2=t1, op0=ALU.mult, op1=ALU.add
            )

        # xq[b, m] = xn[b, :] @ Wx.T[:, m] + bx[m]: xn.T stationary, Wx.T moving
        xq_sb = big.tile([128, H * DK], F32)
        for half in range(2):
            pq = psq.tile([128, 512], F32, tag="psq")
            for j in range(8):
                nc.tensor.matmul(
                    pq,
                    lhsT=xnt[j],
                    rhs=wx[j][:, 512 * half : 512 * (half + 1)],
                    start=(j == 0),
                    stop=False,
                )
            nc.tensor.matmul(
                pq,
                lhsT=onesrow[:, 0:128],
                rhs=bxr[:, 512 * half : 512 * (half + 1)],
                start=False,
                stop=True,
            )
            nc.scalar.copy(out=xq_sb[:, 512 * half : 512 * (half + 1)], in_=pq)

        # transpose per head-pair: xq2T[dkq, h*B + b] = xq[b, h*DK + dkq]
        xq2T = big.tile([64, H * B], F32)
        for i in range(8):
            ptp = ps.tile([128, 128], F32, tag="ps")
            nc.tensor.transpose(ptp, xq_sb[:, 128 * i : 128 * (i + 1)], ident)
            nc.scalar.copy(out=xq2T[:, 256 * i : 256 * i + 128], in_=ptp[0:64, :])
            nc.scalar.copy(out=xq2T[:, 256 * i + 128 : 256 * i + 256], in_=ptp[64:128, :])

        # rest of the k stream
        for t in range(6, NT):
            ktile(t)

        # BN2 stats over (b, h) per d2, computed in the [b, m] layout:
        # column sums of xq_sb/xq_sb^2 via ones-matmuls, then fold h on 1 lane
        srow = stat.tile([1, 2, H * DK], F32)
        for half in range(2):
            sq_sb = big.tile([128, 512], F32, tag="sqsb", name=f"sq{half}", bufs=1)
            nc.scalar.activation(
                out=sq_sb, in_=xq_sb[:, 512 * half : 512 * (half + 1)],
                func=AF.Square, bias=0.0, scale=1.0,
            )
            psSh = ps.tile([1, 2, 512], F32, tag="psS", bufs=1, name=f"psS{half}")
            nc.tensor.matmul(
                psSh[:, 0, :], lhsT=ones128,
                rhs=xq_sb[:, 512 * half : 512 * (half + 1)], start=True, stop=True,
            )
            nc.tensor.matmul(psSh[:, 1, :], lhsT=ones128, rhs=sq_sb, start=True, stop=True)
            nc.scalar.copy(out=srow[:, :, 512 * half : 512 * (half + 1)], in_=psSh)
        # fold h (stride-DK view, innermost = h) -> [1, 2, DK] sums, then *1/(H*B)
        mrow = stat.tile([1, 2, DK], F32)
        nc.vector.tensor_reduce(
            out=mrow,
            in_=srow.rearrange("p s (h d) -> p s d h", h=H),
            axis=mybir.AxisListType.X, op=ALU.add,
        )
        nc.scalar.mul(out=mrow, in_=mrow, mul=1.0 / (H * B))
        # var = E[x^2] - E[x]^2 on the single lane
        vrow = stat.tile([1, DK], F32)
        nc.vector.tensor_mul(out=vrow, in0=mrow[:, 0, :], in1=mrow[:, 0, :])
        nc.vector.tensor_sub(out=vrow, in0=mrow[:, 1, :], in1=vrow)
        # transpose mean/var rows -> per-partition [64, 1] scalars
        psmv = ps.tile([64, 2], F32, tag="psmv", bufs=1)
        nc.tensor.transpose(psmv[:, 0:1], mrow[:, 0, :], ident[0:1, 0:1])
        nc.tensor.transpose(psmv[:, 1:2], vrow, ident[0:1, 0:1])
        mv2 = stat.tile([64, 2], F32)
        nc.scalar.copy(out=mv2, in_=psmv)
        rs2 = stat.tile([64, 1], F32)
        nc.scalar.activation(
            out=rs2, in_=mv2[:, 1:2], func=AF.Sqrt, bias=epst[0:64, :], scale=1.0
        )
        nc.vector.reciprocal(out=rs2, in_=rs2)
        s2 = stat.tile([64, 1], F32)
        nc.vector.tensor_mul(out=s2, in0=rs2, in1=vp64_sb[:, 0:1])
        t2 = stat.tile([64, 1], F32)
        nc.vector.tensor_mul(out=t2, in0=mv2[:, 0:1], in1=s2)
        nc.vector.tensor_sub(out=t2, in0=vp64_sb[:, 1:2], in1=t2)
        # apply on ACT (per-partition scale+bias), keeping DVE free for the k adds
        nc.scalar.activation(out=xq2T, in_=xq2T, func=AF.Identity, bias=t2, scale=s2)

        # latent in head-pair layout: latp[hl*64+d2, i*128+b] = latent[(2i+hl)*B+b, d2]
        latp = big.tile([128, 8 * B], F32)
        for half in range(2):
            psl = psq.tile([128, 512], F32, tag="psq")
            for hh in range(4):
                h = half * 8 + hh * 2
                for hl in range(2):
                    nc.tensor.matmul(
                        psl[64 * hl : 64 * hl + 64, 128 * hh : 128 * (hh + 1)],
                        lhsT=wlin,
                        rhs=xq2T[:, B * (h + hl) : B * (h + hl + 1)],
                        start=True,
                        stop=True,
                    )
            # blin (duplicated per half-pair) via ACT copy bias
            nc.scalar.activation(
                out=latp[:, 512 * half : 512 * (half + 1)],
                in_=psl,
                func=AF.Identity,
                bias=vp128_sb[:, 16:17],
                scale=1.0,
            )

        # out3[b, o] = sum_i latp[:, i*128:+128].T @ WoT[128i:+128, :] + bo
        ps3 = psq.tile([128, DO], F32, tag="psq")
        for i in range(8):
            nc.tensor.matmul(
                ps3,
                lhsT=latp[:, 128 * i : 128 * (i + 1)],
                rhs=wop[i],
                start=(i == 0),
                stop=False,
            )
        nc.tensor.matmul(ps3, lhsT=onesrow[:, 0:128], rhs=bor, start=False, stop=True)
        out3_sb = outp.tile([128, DO], F32, tag="out3sb")
        nc.scalar.copy(out=out3_sb, in_=ps3)
        nc.scalar.dma_start(out=out3_o, in_=out3_sb)

        # fold ns per batch: ksumT[dk, b] = sum_p (accD+accG)[p, dk] * G[p, b]
        psT = psq.tile([128, 4, BL], F32, tag="psq")
        for g in range(4):
            nc.tensor.matmul(
                psT[:, g, :], lhsT=accD[:, 128 * g : 128 * (g + 1)], rhs=g_sb,
                start=True, stop=False,
            )
            nc.tensor.matmul(
                psT[:, g, :], lhsT=accG[:, 128 * g : 128 * (g + 1)], rhs=g_sb,
                start=False, stop=True,
            )
        ksumT = big.tile([128, 4, BL], F32)
        nc.scalar.copy(out=ksumT, in_=psT)

        # out2 = ksum @ Wko.T + bko
        ps2 = psq.tile([BL, DO], F32, tag="psq")
        for g in range(4):
            nc.tensor.matmul(
                ps2, lhsT=ksumT[:, g, :], rhs=wko[g], start=(g == 0), stop=False
            )
        nc.tensor.matmul(ps2, lhsT=onesrow[:, 0:BL], rhs=bkor, start=False, stop=True)
        out2_sb = outp.tile([BL, DO], F32, tag="out2sb")
        nc.scalar.copy(out=out2_sb, in_=ps2)
        nc.scalar.dma_start(out=out2_o, in_=out2_sb)

    nc.compile()
    return nc


_CACHE = {"nc": None}


def get_nc():
    if _CACHE["nc"] is None:
        _CACHE["nc"] = build_program()
    return _CACHE["nc"]


def make_in_maps(inputs):
    f = lambda a: np.ascontiguousarray(np.asarray(a, dtype=np.float32))
    x, k = f(inputs["x"]), np.asarray(inputs["k"], dtype=np.float32)
    bn1_g, bn1_b = f(inputs["bn1_g"]), f(inputs["bn1_b"])
    Wx, bx = f(inputs["Wx"]), f(inputs["bx"])
    Wk, bk = f(inputs["Wk"]), f(inputs["bk"])
    bn2_g, bn2_b = f(inputs["bn2_g"]), f(inputs["bn2_b"])
    Wlin, blin = f(inputs["Wlin"]), f(inputs["blin"])
    Wo, bo = f(inputs["Wo"]), f(inputs["bo"])

    bko = (np.float32(NK) * (Wo @ bk) + bo).astype(np.float32)
    vp128v = np.zeros((128, 17), np.float32)
    vp128v[:, 0:8] = bn1_g.reshape(8, 128).T
    vp128v[:, 8:16] = bn1_b.reshape(8, 128).T
    vp128v[:, 16] = np.concatenate([blin, blin])
    vp64v = np.zeros((64, 2), np.float32)
    vp64v[:, 0] = bn2_g
    vp64v[:, 1] = bn2_b
    gmatv = (np.arange(128)[:, None] // 8 == np.arange(BL)[None, :]).astype(np.float32)

    shared = {
        "xT": np.ascontiguousarray(x.T),
        "WxT": np.ascontiguousarray(Wx.T),
        "WlinT": np.ascontiguousarray(Wlin.T),
        "WoT": np.ascontiguousarray(Wo.T),
        "WkoT": np.ascontiguousarray((Wo @ Wk).T),
        "gmat": gmatv,
        "vp128": vp128v,
        "vp64": vp64v,
        "bx_row": bx.reshape(1, H * DK),
        "bo_row": bo.reshape(1, DO),
        "bko_row": bko.reshape(1, DO),
    }
    return [
        {**shared, "kc": np.ascontiguousarray(k[c * BL : (c + 1) * BL])}
        for c in range(NCORES)
    ]


def gather_outputs(results):
    attn = np.concatenate(
        [r["attn_o"].reshape(BL, H, NK) for r in results], axis=0
    ).astype(np.float32)
    out2 = np.concatenate([r["out2_o"] for r in results], axis=0).astype(np.float32)
    out3 = results[0]["out3_o"].astype(np.float32)
    return attn, out2, out3


def kernel(**inputs):
    nc = get_nc()
    in_maps = make_in_maps(inputs)
    res = run_bass_kernel_spmd(nc, in_maps, core_ids=list(range(NCORES)))
    return gather_outputs(res.results)
